# revision 4
# baseline (speedup 1.0000x reference)
"""ChamferLoss Trainium2 kernel (v3 — hardware-loop structured, consolidated).

Data-parallel over batch: 16 batches / 8 cores = 2 each.
  m[b,i,j] = -pdist = 2 x_i.y_j - ||x_i||^2 - ||y_j||^2   (first 3 channels)
  loss = -( mean_bi max_j m + mean_bj max_i m )

The cross term comes from a single K=13 bf16 augmented matmul (hi/lo split
gives fp32-class accuracy at bf16 PE speed):
  x-side rows: [xh(3), xh(3), xl(3), -rxh, -rxl, -1, -1]
  y-side rows: [Yh(3), Yl(3), Yh(3),  1,    1,  Ryh, Ryl],  Y = 2y, Ry=||y||^2
The augmented operands are built ON THE HOST (exact fp32->bf16 hi/lo split,
channel-major) and shipped as one contiguous [13, 4, 4096] bf16 tensor, so
the device does zero prep work (no PE transposes, no DVE augmentation).

Measured cost model for this axon/PJRT execution path (probes*.py):
  - every *static* instruction costs a serialized tax (~43-46 us for
    matmul/DVE/DMA class) regardless of operand size; engines don't overlap;
  - a tc.For_i hardware loop body pays the tax once; iterations are cheap
    (<10 us) but each For_i instance costs ~1.1 ms of fixed machinery;
  - gpsimd.tensor_reduce(axis=C) is slow; partition_all_reduce is ~2x
    cheaper for the column finals.
Structure: ONE 32-iteration For_i; body = one combined weight-stage copy
(ldweights can't take register offsets) + per batch: 8x512-wide K=13 bf16
matmuls filling [128,4096] f32 PSUM + tensor_reduce(X,max) into a rowpart
column + tensor_tensor max into colacc. Finals: one partition_all_reduce
over the combined [128, 2*4096] colacc + two 3D-AP reduces into a [128,4]
partial tile; the host does the final gather-sum.
"""

from contextlib import ExitStack

import numpy as np

import concourse.bass as bass
import concourse.bacc as bacc
import concourse.tile as tile
from concourse import bass_isa, mybir
from concourse.bass import ds
from concourse.bass_utils import run_bass_kernel_spmd

F32 = mybir.dt.float32
BF16 = mybir.dt.bfloat16
AX = mybir.AxisListType
OP = mybir.AluOpType

NEG_BIG = -3.0e38

B_FULL = 16
N_FULL = 4096
C_FULL = 6
N_CORES = 8
KAUG = 13


def build_nc(b_loc=2, n=4096, c_in=6, num_devices=8, reps=1):
    """Per-core program. Input aug: [13, 2*b_loc, n] bf16 (host-prepped
    augmented operands: cols 0..b_loc-1 = x-side, b_loc..2*b_loc-1 = y-side);
    output "partial" [128, 2*b_loc] f32:
      partial[:, 0:b_loc]      = per-partition sums of rowmax (x side)
      partial[0, b_loc + b]    = total colmax sum (y side), rest zeros.
    Host computes loss = -partial.sum()/ (B*N) over all cores.
    """
    NP = 128
    NQ = n // NP                  # row-tiles per batch (32)
    NS = n // 512                 # 512-wide matmul slabs (8)

    nc = bacc.Bacc(
        "TRN2",
        target_bir_lowering=False,
        debug=False,
        enable_asserts=False,
        num_devices=num_devices,
    )

    aug_d = nc.declare_dram_parameter(
        "aug", [KAUG, 2 * b_loc, n], BF16, isOutput=False
    ).ap()
    out_d = nc.declare_dram_parameter(
        "partial", [NP, 2 * b_loc], F32, isOutput=True
    ).ap()

    with tile.TileContext(nc) as tc, ExitStack() as ctx:
        singles = ctx.enter_context(tc.tile_pool(name="singles", bufs=1))
        psum_pool = ctx.enter_context(tc.tile_pool(name="psum", bufs=1, space="PSUM"))

        def emit_body():
            aug_s = singles.tile([KAUG, 2 * b_loc, n], BF16, tag="aug", name="aug_s")
            nc.sync.dma_start(out=aug_s, in_=aug_d)

            colacc = singles.tile([NP, b_loc, n], F32, tag="colacc", name="colacc")
            rowpart = singles.tile([NP, b_loc * NQ], F32, tag="rowpart",
                                   name="rowpart")
            sums = singles.tile([NP, 2 * b_loc], F32, tag="sums", name="sums")
            cm = singles.tile([NP, b_loc * n], F32, tag="cm", name="cm")
            nc.vector.memset(colacc, NEG_BIG)
            nc.vector.memset(sums, 0.0)

            ps = psum_pool.tile([NP, n], F32, tag="ps", name="ps_main")
            # ldweights can't take register offsets: stage both batches'
            # row-tiles of weights through one fixed-address tile.
            wt = singles.tile([KAUG, b_loc, NP], BF16, tag="wt", name="wt")

            with tc.For_i(0, NQ, 1) as ri:
                nc.vector.tensor_copy(wt, aug_s[:, 0:b_loc, ds(ri * NP, NP)])
                for b in range(b_loc):
                    for s in range(NS):
                        nc.tensor.matmul(
                            ps[:, s * 512:(s + 1) * 512],
                            lhsT=wt[:, b, :],
                            rhs=aug_s[:, b_loc + b, s * 512:(s + 1) * 512],
                            start=True,
                            stop=True,
                        )
                    nc.vector.tensor_reduce(
                        rowpart[:, ds(b * NQ + ri, 1)], ps, axis=AX.X, op=OP.max
                    )
                    nc.vector.tensor_tensor(
                        colacc[:, b, :], colacc[:, b, :], ps, op=OP.max
                    )

            # finals: row side = sum of per-tile rowmaxes; col side = sum of
            # per-column maxes (partition reduce via gpsimd all-reduce).
            nc.vector.tensor_reduce(
                sums[:, 0:b_loc], rowpart.rearrange("p (b q) -> p b q", b=b_loc),
                axis=AX.X, op=OP.add,
            )
            nc.gpsimd.partition_all_reduce(
                cm, colacc.rearrange("p b n -> p (b n)"), NP,
                bass_isa.ReduceOp.max,
            )
            nc.vector.tensor_reduce(
                sums[0:1, b_loc:2 * b_loc],
                cm[0:1, :].rearrange("p (b n) -> p b n", b=b_loc),
                axis=AX.X, op=OP.add,
            )
            nc.sync.dma_start(out=out_d, in_=sums)

        for _ in range(reps):
            emit_body()

    nc.compile()
    return nc


def _host_aug(x: np.ndarray, y: np.ndarray) -> np.ndarray:
    """Build the augmented [13, 2*b, n] bf16 operand block for one core.

    x, y: [b, n, 6] f32.  Coordinate channels are the first 3.
    """
    import ml_dtypes

    b, n, _ = x.shape
    xc = np.ascontiguousarray(x[:, :, :3]).astype(np.float32)   # [b, n, 3]
    yc = np.ascontiguousarray(y[:, :, :3]).astype(np.float32)

    def split(v):
        hi = v.astype(ml_dtypes.bfloat16).astype(np.float32)
        lo = (v - hi).astype(ml_dtypes.bfloat16).astype(np.float32)
        return hi, lo

    xh, xl = split(xc)                                  # [b, n, 3]
    rx = np.sum(xc * xc, axis=-1)                       # [b, n]
    rxh, rxl = split(rx)

    Y = 2.0 * yc
    Yh, Yl = split(Y)
    ry = np.sum(yc * yc, axis=-1)
    ryh, ryl = split(ry)

    ones = np.ones_like(rx)

    # x-side rows (K=13): [xh(3), xh(3), xl(3), -rxh, -rxl, -1, -1]
    ax = np.concatenate(
        [xh, xh, xl, -rxh[..., None], -rxl[..., None],
         -ones[..., None], -ones[..., None]], axis=-1)   # [b, n, 13]
    # y-side rows: [Yh(3), Yl(3), Yh(3), 1, 1, ryh, ryl]
    ay = np.concatenate(
        [Yh, Yl, Yh, ones[..., None], ones[..., None],
         ryh[..., None], ryl[..., None]], axis=-1)       # [b, n, 13]

    # -> [13, 2b, n] channel-major
    aug = np.empty((KAUG, 2 * b, n), dtype=ml_dtypes.bfloat16)
    for bi in range(b):
        aug[:, bi, :] = ax[bi].T.astype(ml_dtypes.bfloat16)
        aug[:, b + bi, :] = ay[bi].T.astype(ml_dtypes.bfloat16)
    return aug


_CACHE = {}


def _get_nc():
    if "nc" not in _CACHE:
        _CACHE["nc"] = build_nc(
            b_loc=B_FULL // N_CORES, n=N_FULL, c_in=C_FULL, num_devices=N_CORES
        )
    return _CACHE["nc"]


def make_in_maps(x: np.ndarray, y: np.ndarray):
    bl = B_FULL // N_CORES
    return [
        {"aug": _host_aug(x[i * bl:(i + 1) * bl], y[i * bl:(i + 1) * bl])}
        for i in range(N_CORES)
    ]


def kernel(x: np.ndarray, y: np.ndarray) -> np.ndarray:
    x = np.ascontiguousarray(np.asarray(x, dtype=np.float32))
    y = np.ascontiguousarray(np.asarray(y, dtype=np.float32))
    assert x.shape == (B_FULL, N_FULL, C_FULL), x.shape
    nc = _get_nc()
    in_maps = make_in_maps(x, y)
    res = run_bass_kernel_spmd(nc, in_maps, list(range(N_CORES)))
    total = sum(float(r["partial"].astype(np.float64).sum()) for r in res.results)
    loss = -total / float(B_FULL * N_FULL)
    return np.float32(loss)


# revision 9
# speedup vs baseline: 1.6395x; 1.6395x over previous
"""ChamferLoss Trainium2 kernel (v3 — hardware-loop structured, consolidated).

Data-parallel over batch: 16 batches / 8 cores = 2 each.
  m[b,i,j] = -pdist = 2 x_i.y_j - ||x_i||^2 - ||y_j||^2   (first 3 channels)
  loss = -( mean_bi max_j m + mean_bj max_i m )

The cross term comes from a single K=13 bf16 augmented matmul (hi/lo split
gives fp32-class accuracy at bf16 PE speed):
  x-side rows: [xh(3), xh(3), xl(3), -rxh, -rxl, -1, -1]
  y-side rows: [Yh(3), Yl(3), Yh(3),  1,    1,  Ryh, Ryl],  Y = 2y, Ry=||y||^2
The augmented operands are built ON THE HOST (exact fp32->bf16 hi/lo split,
channel-major) and shipped as one contiguous [13, 4, 4096] bf16 tensor, so
the device does zero prep work (no PE transposes, no DVE augmentation).

Measured cost model for this axon/PJRT execution path (probes*.py):
  - every *static* instruction costs a serialized tax (~43-46 us for
    matmul/DVE/DMA class) regardless of operand size; engines don't overlap;
  - a tc.For_i hardware loop body pays the tax once; iterations are cheap
    (<10 us) but each For_i instance costs ~1.1 ms of fixed machinery;
  - gpsimd.tensor_reduce(axis=C) is slow; partition_all_reduce is ~2x
    cheaper for the column finals.
Structure: ONE 32-iteration For_i; body = one combined weight-stage copy
(ldweights can't take register offsets) + per batch: 8x512-wide K=13 bf16
matmuls filling [128,4096] f32 PSUM + tensor_reduce(X,max) into a rowpart
column + tensor_tensor max into colacc. Finals: one partition_all_reduce
over the combined [128, 2*4096] colacc + two 3D-AP reduces into a [128,4]
partial tile; the host does the final gather-sum.
"""

from contextlib import ExitStack

import numpy as np

import concourse.bass as bass
import concourse.bacc as bacc
import concourse.tile as tile
from concourse import bass_isa, mybir
from concourse.bass import ds
from concourse.bass_utils import run_bass_kernel_spmd

F32 = mybir.dt.float32
BF16 = mybir.dt.bfloat16
AX = mybir.AxisListType
OP = mybir.AluOpType

NEG_BIG = -3.0e38

B_FULL = 16
N_FULL = 4096
C_FULL = 6
N_CORES = 8
KAUG = 13

# col-side finals strategy: "par" = gpsimd partition_all_reduce on device,
# "par_bf16" = same but colacc kept in bf16 (half the gpsimd traffic),
# "host" = DMA colacc out and reduce on host.
FINALS = "par"


def build_nc(b_loc=2, n=4096, c_in=6, num_devices=8, reps=1, finals=None):
    """Per-core program. Input aug: [13, 2*b_loc, n] bf16 (host-prepped
    augmented operands: cols 0..b_loc-1 = x-side, b_loc..2*b_loc-1 = y-side);
    output "partial" [128, 2*b_loc] f32:
      partial[:, 0:b_loc]      = per-partition sums of rowmax (x side)
      partial[0, b_loc + b]    = total colmax sum (y side), rest zeros.
    Host computes loss = -partial.sum()/ (B*N) over all cores.
    """
    NP = 128
    NQ = n // NP                  # row-tiles per batch (32)
    NS = n // 512                 # 512-wide matmul slabs (8)
    if finals is None:
        finals = FINALS
    cdt = BF16 if finals == "par_bf16" else F32

    nc = bacc.Bacc(
        "TRN2",
        target_bir_lowering=False,
        debug=False,
        enable_asserts=False,
        num_devices=num_devices,
    )

    aug_d = nc.declare_dram_parameter(
        "aug", [KAUG, 2 * b_loc, n], BF16, isOutput=False
    ).ap()
    out_d = nc.declare_dram_parameter(
        "partial", [NP, 2 * b_loc], F32, isOutput=True
    ).ap()
    col_d = None
    if finals == "host":
        col_d = nc.declare_dram_parameter(
            "colacc", [NP, b_loc, n], F32, isOutput=True
        ).ap()

    with tile.TileContext(nc) as tc, ExitStack() as ctx:
        singles = ctx.enter_context(tc.tile_pool(name="singles", bufs=1))
        psum_pool = ctx.enter_context(tc.tile_pool(name="psum", bufs=1, space="PSUM"))

        def emit_body():
            aug_s = singles.tile([KAUG, 2 * b_loc, n], BF16, tag="aug", name="aug_s")
            nc.sync.dma_start(out=aug_s, in_=aug_d)

            colacc = singles.tile([NP, b_loc, n], cdt, tag="colacc", name="colacc")
            rowpart = singles.tile([NP, b_loc * NQ], F32, tag="rowpart",
                                   name="rowpart")
            sums = singles.tile([NP, 2 * b_loc], F32, tag="sums", name="sums")
            cm = singles.tile([NP, b_loc * n], F32, tag="cm", name="cm")
            nc.vector.memset(colacc, NEG_BIG)
            nc.vector.memset(sums, 0.0)

            ps = psum_pool.tile([NP, n], F32, tag="ps", name="ps_main")
            # ldweights can't take register offsets: stage both batches'
            # row-tiles of weights through one fixed-address tile.
            wt = singles.tile([KAUG, b_loc, NP], BF16, tag="wt", name="wt")

            with tc.For_i(0, NQ, 1) as ri:
                nc.vector.tensor_copy(wt, aug_s[:, 0:b_loc, ds(ri * NP, NP)])
                for b in range(b_loc):
                    for s in range(NS):
                        nc.tensor.matmul(
                            ps[:, s * 512:(s + 1) * 512],
                            lhsT=wt[:, b, :],
                            rhs=aug_s[:, b_loc + b, s * 512:(s + 1) * 512],
                            start=True,
                            stop=True,
                        )
                    nc.vector.tensor_reduce(
                        rowpart[:, ds(b * NQ + ri, 1)], ps, axis=AX.X, op=OP.max
                    )
                    nc.vector.tensor_tensor(
                        colacc[:, b, :], colacc[:, b, :], ps, op=OP.max
                    )

            # finals: row side = sum of per-tile rowmaxes; col side = sum of
            # per-column maxes (partition reduce via gpsimd all-reduce, or
            # shipped to the host).
            nc.vector.tensor_reduce(
                sums[:, 0:b_loc], rowpart.rearrange("p (b q) -> p b q", b=b_loc),
                axis=AX.X, op=OP.add,
            )
            if finals == "host":
                nc.sync.dma_start(out=col_d, in_=colacc)
            else:
                nc.gpsimd.partition_all_reduce(
                    cm, colacc.rearrange("p b n -> p (b n)"), NP,
                    bass_isa.ReduceOp.max,
                )
                nc.vector.tensor_reduce(
                    sums[0:1, b_loc:2 * b_loc],
                    cm[0:1, :].rearrange("p (b n) -> p b n", b=b_loc),
                    axis=AX.X, op=OP.add,
                )
            nc.sync.dma_start(out=out_d, in_=sums)

        for _ in range(reps):
            emit_body()

    nc.compile()
    return nc


def _host_aug(x: np.ndarray, y: np.ndarray) -> np.ndarray:
    """Build the augmented [13, 2*b, n] bf16 operand block for one core.

    x, y: [b, n, 6] f32.  Coordinate channels are the first 3.
    """
    import ml_dtypes

    b, n, _ = x.shape
    xc = np.ascontiguousarray(x[:, :, :3]).astype(np.float32)   # [b, n, 3]
    yc = np.ascontiguousarray(y[:, :, :3]).astype(np.float32)

    def split(v):
        hi = v.astype(ml_dtypes.bfloat16).astype(np.float32)
        lo = (v - hi).astype(ml_dtypes.bfloat16).astype(np.float32)
        return hi, lo

    xh, xl = split(xc)                                  # [b, n, 3]
    rx = np.sum(xc * xc, axis=-1)                       # [b, n]
    rxh, rxl = split(rx)

    Y = 2.0 * yc
    Yh, Yl = split(Y)
    ry = np.sum(yc * yc, axis=-1)
    ryh, ryl = split(ry)

    ones = np.ones_like(rx)

    # x-side rows (K=13): [xh(3), xh(3), xl(3), -rxh, -rxl, -1, -1]
    ax = np.concatenate(
        [xh, xh, xl, -rxh[..., None], -rxl[..., None],
         -ones[..., None], -ones[..., None]], axis=-1)   # [b, n, 13]
    # y-side rows: [Yh(3), Yl(3), Yh(3), 1, 1, ryh, ryl]
    ay = np.concatenate(
        [Yh, Yl, Yh, ones[..., None], ones[..., None],
         ryh[..., None], ryl[..., None]], axis=-1)       # [b, n, 13]

    # -> [13, 2b, n] channel-major
    aug = np.empty((KAUG, 2 * b, n), dtype=ml_dtypes.bfloat16)
    for bi in range(b):
        aug[:, bi, :] = ax[bi].T.astype(ml_dtypes.bfloat16)
        aug[:, b + bi, :] = ay[bi].T.astype(ml_dtypes.bfloat16)
    return aug


_CACHE = {}


def _get_nc():
    if "nc" not in _CACHE:
        _CACHE["nc"] = build_nc(
            b_loc=B_FULL // N_CORES, n=N_FULL, c_in=C_FULL, num_devices=N_CORES
        )
    return _CACHE["nc"]


def make_in_maps(x: np.ndarray, y: np.ndarray):
    bl = B_FULL // N_CORES
    return [
        {"aug": _host_aug(x[i * bl:(i + 1) * bl], y[i * bl:(i + 1) * bl])}
        for i in range(N_CORES)
    ]


def gather_loss(res) -> float:
    total = 0.0
    for r in res.results:
        total += float(r["partial"].astype(np.float64).sum())
        if "colacc" in r:
            # host-side column finals: max over the 128 partition rows
            ca = r["colacc"].astype(np.float64)      # [128, b_loc, n]
            total += float(ca.max(axis=0).sum())
    return -total / float(B_FULL * N_FULL)


def kernel(x: np.ndarray, y: np.ndarray) -> np.ndarray:
    x = np.ascontiguousarray(np.asarray(x, dtype=np.float32))
    y = np.ascontiguousarray(np.asarray(y, dtype=np.float32))
    assert x.shape == (B_FULL, N_FULL, C_FULL), x.shape
    nc = _get_nc()
    in_maps = make_in_maps(x, y)
    res = run_bass_kernel_spmd(nc, in_maps, list(range(N_CORES)))
    return np.float32(gather_loss(res))
